# revision 24
# baseline (speedup 1.0000x reference)
"""Trainium2 Bass kernel for masked multi-head attention (nn_Attention_25271587569919).

Problem: B=4, S=2048, D=1024, 16 heads x 64. out = (softmax(QK^T/8 + pad/causal mask) V) WO.

Sharding: 8 cores = 4 batches x 2 head-groups (Megatron-style). Each core computes,
for its batch b and its 8 heads:
    QT/KT = (X Wq_g)^T in [dh, seq] layout,  V = X Wv_g in [seq, dh] layout,
    S^T tiles [k, q] (so pad mask = per-partition ACT bias, no transposes anywhere),
    P = exp(S^T/8 + pad) (no max subtraction; scores are O(1)),
    causal handled by narrowing the q-range of diagonal-straddling tiles plus one
    128x128 triangular 0/1 multiply on the diagonal block,
    ctx^T (+rowsum via an all-ones 65th column of V) = Vaug^T @ P accumulated in PSUM,
    ctx^T normalized by 1/rowsum (reciprocal of the rowsum row, broadcast across
    partitions with a K=1 ones-matmul), then out_partial = ctx @ Wo_g.
Host sums the two head-group partials per batch. No collectives needed.

The emission is software-pipelined over q-stripes j: projection work for stripe
j+1 and the output projection for stripe j-1 are interleaved between the
attention i-iterations of stripe j, and scores for i+1 are emitted before the
PV matmuls of i so the TensorEngine never sits behind the exp latency.
"""

import numpy as np
import ml_dtypes

BF = ml_dtypes.bfloat16
S = 2048
D = 1024
HG = 512          # head-group width (8 heads x 64)
DH = 64
NKT = 16          # seq tiles of 128 (k side)
NQT = 4           # seq tiles of 512 (q side)
NEG = -30000.0

_CACHE = {}


def _build():
    import concourse.bass as bass  # noqa: F401
    import concourse.tile as tile
    from concourse import bacc, mybir

    f32 = mybir.dt.float32
    bf16 = mybir.dt.bfloat16
    Exp = mybir.ActivationFunctionType.Exp

    nc = bacc.Bacc("TRN2", target_bir_lowering=False, debug=False, num_devices=8)

    xq_d = nc.dram_tensor("xq", [D, S], bf16, kind="ExternalInput")
    xk_d = nc.dram_tensor("xk", [D, S], bf16, kind="ExternalInput")
    xv_d = nc.dram_tensor("xv", [D, S], bf16, kind="ExternalInput")
    wq_d = nc.dram_tensor("wq", [D, HG], bf16, kind="ExternalInput")
    wk_d = nc.dram_tensor("wk", [D, HG], bf16, kind="ExternalInput")
    wv_d = nc.dram_tensor("wv", [D, HG], bf16, kind="ExternalInput")
    wo_d = nc.dram_tensor("wo", [HG, D], bf16, kind="ExternalInput")
    padb_d = nc.dram_tensor("padb", [128, NKT], f32, kind="ExternalInput")
    trim_d = nc.dram_tensor("trim", [128, 128], bf16, kind="ExternalInput")
    out_d = nc.dram_tensor("out", [S, D], f32, kind="ExternalOutput")

    with tile.TileContext(nc) as tc:
        with (
            tc.tile_pool(name="consts", bufs=1) as consts,
            tc.tile_pool(name="big", bufs=1) as big,
            tc.tile_pool(name="xpool", bufs=6) as xpool,
            tc.tile_pool(name="ppool", bufs=3) as ppool,
            tc.tile_pool(name="cspool", bufs=4) as cspool,
            tc.tile_pool(name="rspool", bufs=4) as rspool,
            tc.tile_pool(name="tmppool", bufs=3) as tmppool,
            tc.tile_pool(name="outpool", bufs=2) as outpool,
            tc.tile_pool(name="pspool", bufs=2, space="PSUM") as pspool,
            tc.tile_pool(name="psupool", bufs=2, space="PSUM") as psupool,
            tc.tile_pool(name="ctxpool", bufs=2, space="PSUM") as ctxpool,
        ):
            wq_sb = consts.tile([128, 8, HG], bf16, tag="wq")
            wk_sb = consts.tile([128, 8, HG], bf16, tag="wk")
            wv_sb = consts.tile([128, 8, HG], bf16, tag="wv")
            wo_sb = consts.tile([128, 4, D], bf16, tag="wo")
            padb_sb = consts.tile([128, NKT], f32, tag="padb")
            trim_sb = consts.tile([128, 128], bf16, tag="trim")
            ones_sb = consts.tile([128, 64], bf16, tag="ones")

            nc.sync.dma_start(out=wq_sb, in_=wq_d.ap().rearrange("(kt p) n -> p kt n", p=128))
            nc.sync.dma_start(out=padb_sb, in_=padb_d.ap())
            nc.sync.dma_start(out=trim_sb, in_=trim_d.ap())
            nc.vector.memset(ones_sb, 1.0)

            def load_w(dst, dram):
                def emit():
                    nc.sync.dma_start(
                        out=dst, in_=dram.ap().rearrange("(kt p) n -> p kt n", p=128))
                return emit

            qt_sb = big.tile([128, 4, S], bf16, tag="qt")    # (X Wq)^T : rows = dh
            kt_sb = big.tile([128, 4, S], bf16, tag="kt")
            vaug_sb = big.tile([128, NKT, 8 * 65], bf16, tag="vaug")  # V + ones col
            ctxt_sb = big.tile([128, 4, S], bf16, tag="ctxt")
            vaug_h = vaug_sb.rearrange("p m (h e) -> p m h e", e=65)
            nc.vector.memset(vaug_h[:, :, :, 64:65], 1.0)

            # ---------- stripe-A: projections for q/seq stripe j ----------
            def a_units(j, first=False):
                qs = slice(512 * j, 512 * (j + 1))
                st = {}

                def load_x(name, dram):
                    def emit():
                        t = xpool.tile([128, 8, 512], bf16, tag="x")
                        nc.sync.dma_start(
                            out=t,
                            in_=dram.ap().rearrange("(kt p) s -> p kt s", p=128)[:, :, qs])
                        st[name] = t
                    return emit

                def proj_t(w_sb, dst_sb, t, x_name):
                    def emit():
                        ps = psupool.tile([128, 512], f32, tag="psu")
                        for kt in range(8):
                            nc.tensor.matmul(
                                ps,
                                lhsT=w_sb[:, kt, 128 * t:128 * (t + 1)],
                                rhs=st[x_name][:, kt, :],
                                start=(kt == 0), stop=(kt == 7),
                            )
                        nc.vector.tensor_copy(out=dst_sb[:, t, qs], in_=ps)
                    return emit

                def proj_v(m):
                    def emit():
                        ps = psupool.tile([128, 512], f32, tag="psu")
                        for kt in range(8):
                            nc.tensor.matmul(
                                ps,
                                lhsT=st["xv"][:, kt, 128 * (m - 4 * j):128 * (m - 4 * j + 1)],
                                rhs=wv_sb[:, kt, :],
                                start=(kt == 0), stop=(kt == 7),
                            )
                        nc.vector.tensor_copy(
                            out=vaug_h[:, m, :, 0:64],
                            in_=ps.rearrange("p (h e) -> p h e", e=64),
                        )
                    return emit

                early = [load_x("xq", xq_d)]
                if first:
                    early.append(load_w(wk_sb, wk_d))
                    early.append(load_w(wv_sb, wv_d))
                early.append(proj_t(wq_sb, qt_sb, 0, "xq"))
                early.append(load_x("xk", xk_d))
                if first:
                    early.append(load_w(wo_sb, wo_d))
                early.append(proj_t(wk_sb, kt_sb, 0, "xk"))
                early.append(load_x("xv", xv_d))
                for m in range(4 * j, 4 * j + 4):
                    early.append(proj_v(m))
                late = []
                for t in range(1, 4):
                    late.append(proj_t(wq_sb, qt_sb, t, "xq"))
                    late.append(proj_t(wk_sb, kt_sb, t, "xk"))
                return early, late

            # ---------- stripe-C: output projection for q stripe j ----------
            def c_units(j):
                def wo_m(m):
                    def emit():
                        o = outpool.tile([128, D], f32, tag="o")
                        for n in range(2):
                            ps = psupool.tile([128, 512], f32, tag="psu")
                            for kt in range(4):
                                nc.tensor.matmul(
                                    ps,
                                    lhsT=ctxt_sb[:, kt, 128 * m:128 * (m + 1)],
                                    rhs=wo_sb[:, kt, 512 * n:512 * (n + 1)],
                                    start=(kt == 0), stop=(kt == 3),
                                )
                            nc.vector.tensor_copy(out=o[:, 512 * n:512 * (n + 1)], in_=ps)
                        nc.gpsimd.dma_start(out=out_d.ap()[128 * m:128 * (m + 1), :], in_=o)
                    return emit
                return [wo_m(m) for m in range(4 * j, 4 * j + 4)]

            # deferred second half of the normalization: broadcast rowsum,
            # reciprocal, scale — scheduled into the NEXT hp block's loop so the
            # broadcast matmul never stalls the PE behind the evacuation copy
            pending_norm = []

            def make_norm2(cs_pair, hp, j):
                qs = slice(512 * j, 512 * (j + 1))

                def emit():
                    for e, cs in cs_pair:
                        bc = psupool.tile([64, 512], f32, tag="psu")
                        nc.tensor.matmul(bc, lhsT=ones_sb[64:65, :],
                                         rhs=cs[64:65, :], start=True, stop=True)
                        rb = rspool.tile([64, 512], f32, tag="rb")
                        nc.vector.reciprocal_approx_fast(rb, bc)
                        if e == 0:
                            nc.vector.tensor_mul(
                                ctxt_sb[0:64, hp, qs], cs[0:64, :], rb)
                        else:
                            tmp = tmppool.tile([64, 512], bf16, tag="tmp")
                            nc.vector.tensor_mul(tmp, cs[0:64, :], rb)
                            nc.gpsimd.dma_start(
                                out=ctxt_sb[64:128, hp, qs], in_=tmp)
                return emit

            # ---------- stripe-B: attention for q stripe j, with interleave ----------
            def b_stripe(j, queue):
                qs = slice(512 * j, 512 * (j + 1))
                ni = 4 * j + 4

                def lo(i):
                    # narrowed column start within the q stripe for straddle tiles
                    return max(0, (i - 4 * j) * 128)

                for hp in range(4):
                    h0, h1 = 2 * hp, 2 * hp + 1
                    ctx_a = ctxpool.tile([65, 512], f32, tag="ctx")
                    ctx_b = ctxpool.tile([65, 512], f32, tag="ctx")
                    sps = {}
                    ps_ = {}

                    def scores(i):
                        c = lo(i)
                        ks = slice(128 * i, 128 * (i + 1))
                        sp = pspool.tile([128, 2, 512], f32, tag="ps")
                        nc.tensor.matmul(
                            sp[:, 0, c:], lhsT=kt_sb[0:64, hp, ks],
                            rhs=qt_sb[0:64, hp, 512 * j + c:512 * (j + 1)],
                            start=True, stop=True)
                        nc.tensor.matmul(
                            sp[:, 1, c:], lhsT=kt_sb[64:128, hp, ks],
                            rhs=qt_sb[64:128, hp, 512 * j + c:512 * (j + 1)],
                            start=True, stop=True)
                        sps[i] = sp

                    scores(0)
                    for i in range(ni):
                        if i + 1 < ni:
                            scores(i + 1)
                        c = lo(i)
                        sp = sps.pop(i)
                        p = ppool.tile([128, 2, 512], bf16, tag="p")
                        nc.scalar.activation(
                            out=p[:, :, c:], in_=sp[:, :, c:], func=Exp,
                            bias=padb_sb[:, i:i + 1], scale=0.125)
                        if i >= 4 * j:
                            nc.vector.tensor_mul(
                                p[:, 0, c:c + 128], p[:, 0, c:c + 128], trim_sb)
                            nc.vector.tensor_mul(
                                p[:, 1, c:c + 128], p[:, 1, c:c + 128], trim_sb)
                        if i == 2 and pending_norm:
                            pending_norm.pop(0)()
                        if queue:
                            # interleaved proj/output work runs on PE while the
                            # ScalarEngine computes this iteration's exp; C units
                            # must not be emitted before pending normalizations
                            # of the stripe they read
                            if queue[0][0] == "c" and pending_norm:
                                pending_norm.pop(0)()
                            else:
                                queue.pop(0)[1]()
                        nc.tensor.matmul(
                            ctx_a[:, c:], lhsT=vaug_sb[:, i, 65 * h0:65 * h0 + 65],
                            rhs=p[:, 0, c:],
                            start=(i == 0), stop=(i == ni - 1))
                        nc.tensor.matmul(
                            ctx_b[:, c:], lhsT=vaug_sb[:, i, 65 * h1:65 * h1 + 65],
                            rhs=p[:, 1, c:],
                            start=(i == 0), stop=(i == ni - 1))
                    cs_pair = []
                    for e, ctx in ((0, ctx_a), (1, ctx_b)):
                        # evacuate PSUM immediately (frees the bank for hp+1)
                        cs = cspool.tile([65, 512], bf16, tag="cs")
                        nc.vector.tensor_copy(out=cs, in_=ctx)
                        cs_pair.append((e, cs))
                    pending_norm.append(make_norm2(cs_pair, hp, j))

            # ---------- emit the pipeline ----------
            a0_early, a0_late = a_units(0, first=True)
            for u in a0_early + a0_late:
                u()
            carry_late = []
            for j in range(NQT):
                queue = []
                if carry_late:
                    queue += [("a", u) for u in carry_late]
                if j + 1 < NQT:
                    early, late = a_units(j + 1)
                    queue += [("a", u) for u in early]
                    carry_late = late
                else:
                    carry_late = []
                if j - 1 >= 0:
                    queue += [("c", u) for u in c_units(j - 1)]
                b_stripe(j, queue)
                for _, u in queue:
                    u()
            while pending_norm:
                pending_norm.pop(0)()
            for u in c_units(NQT - 1):
                u()

    nc.compile()
    return nc


def _make_trim():
    p = np.arange(128)[:, None]
    f = np.arange(128)[None, :]
    return (f >= p).astype(np.float32).astype(BF)


def kernel(Q_emb, K_emb, V_emb, Q_ini, K_ini, WQ, WK, WV, WO):
    from concourse.bass_utils import run_bass_kernel_spmd

    if "nc" not in _CACHE:
        _CACHE["nc"] = _build()
    nc = _CACHE["nc"]

    Q_emb = np.asarray(Q_emb, np.float32)
    K_emb = np.asarray(K_emb, np.float32)
    V_emb = np.asarray(V_emb, np.float32)
    K_ini = np.asarray(K_ini)
    WQ = np.asarray(WQ, np.float32)
    WK = np.asarray(WK, np.float32)
    WV = np.asarray(WV, np.float32)
    WO = np.asarray(WO, np.float32)

    trim = _make_trim()
    in_maps = []
    for c in range(8):
        b, g = c // 2, c % 2
        gs = slice(HG * g, HG * (g + 1))
        padb = np.where(K_ini[b] != 0, 0.0, NEG).astype(np.float32)
        in_maps.append({
            "xq": Q_emb[b].T.astype(BF),
            "xk": K_emb[b].T.astype(BF),
            "xv": V_emb[b].T.astype(BF),
            "wq": WQ[:, gs].astype(BF),
            "wk": WK[:, gs].astype(BF),
            "wv": WV[:, gs].astype(BF),
            "wo": WO[gs, :].astype(BF),
            "padb": padb.reshape(NKT, 128).T.copy(),
            "trim": trim,
        })

    _CACHE["in_maps"] = in_maps
    res = run_bass_kernel_spmd(nc, in_maps, list(range(8)))
    parts = [res.results[c]["out"] for c in range(8)]
    out = np.stack([parts[2 * b] + parts[2 * b + 1] for b in range(4)])
    return out.astype(np.float32)
